# revision 28
# baseline (speedup 1.0000x reference)
"""Trainium2 Bass kernel for nn_BehavioralCircuit — v3 superblock solver.

Reference: T=100000 sequential steps of a reward-modulated Hebbian rule over
512 independent 2-D units:
    r[t] = rewards[t] - movavg10(rewards)[t];  u = LR*r
    h    = sigmoid(W @ x_t);  m[t] = h.mean();  W += u[t] * outer(h, x_t)

Scheme (validated against a numpy mirror; full-run rel err ~6e-3):
  Superblocks of SB=252 steps = 2 tiles (a,b) of TAU=126.  Per sb s,
  one PSUM tile A[s] [128, 128] holds both tiles' pre-activations
  (64 units per column half).  Two wide sigmoids per sb:
    h1(s) = sig(X W(s-2)_ledger + X.(c^T h1)(s-2) + CR(s-1->s) h1(s-1)
               + 0.5-seeded within-coupling bias)        [one 128-wide ACT]
    h2(s) = sig(same + K_within (h1(s) - 0.5))           [one 128-wide ACT]
  The 0.5 seed is folded into a host-precomputed bias row of the tiny
  base matmul; its removal at the h2 stage is folded into row 126 of the
  within matrices (h tiles' row 126 is exactly sig(0)=0.5).
  Cross couplings older than one sb are rank-2 (diag(u) X_src X_tgt^T) and
  fold into an f32 weight ledger via tiny c^T h matmuls; only the 3 within
  mats (fp16) and 4 prev-sb cross mats (fp8e4) are streamed from HBM.
  The only tight serial chain is sig1(s-1) -> 4 CR matmuls -> sig1(s).

Per core: 64 units; 8 cores partition the 512 units; host sums unit means.
"""

import sys

import numpy as np

sys.path.insert(0, "/opt/trn_rl_repo")

import concourse.bass as bass
import concourse.bacc as bacc
import concourse.tile as tile
from concourse import mybir
from concourse.bass_utils import run_bass_kernel_spmd

TAU = 126
MP = 128
G = 2
SB = G * TAU            # 252
T_FULL = 100000
NSB_FULL = (T_FULL + SB - 1) // SB   # 397
NB_FULL = NSB_FULL      # test.py compat (loop count = superblocks)
NH = 512
NCORES = 8
UH = NH // NCORES       # 64
CH = 16                 # superblocks per DMA chunk
LR = 0.1
WINDOW = 10

F32 = mybir.dt.float32
F16 = mybir.dt.float16
F8 = mybir.dt.float8e4
AF = mybir.ActivationFunctionType
OP = mybir.AluOpType


# ---------------------------------------------------------------------------
# Host-side stream preparation (shared across cores)
# ---------------------------------------------------------------------------

def _movavg_u(rewards, t_pad):
    cs = np.cumsum(rewards, dtype=np.float64)
    sh = np.concatenate([np.zeros(WINDOW), cs[:-WINDOW]])
    wsum = cs - sh
    count = np.minimum(np.arange(len(rewards)) + 1.0, float(WINDOW))
    u = (LR * (rewards - wsum / count)).astype(np.float32)
    up = np.zeros((t_pad,), np.float32)
    up[: len(rewards)] = u
    return up


def prep_streams(X, rewards, nsb):
    t_pad = nsb * SB
    Xp = np.zeros((t_pad, 2), np.float32)
    Xp[: X.shape[0]] = X
    up = _movavg_u(rewards, t_pad)
    Xa = Xp.reshape(nsb, SB, 2)[:, :TAU]          # [s, 126, 2]
    Xb = Xp.reshape(nsb, SB, 2)[:, TAU:]
    ua = up.reshape(nsb, SB)[:, :TAU]
    ub = up.reshape(nsb, SB)[:, TAU:]

    strict = np.triu(np.ones((TAU, TAU), np.float32), 1)   # [sp, t]: sp < t

    def cmat(Xs, us, Xt, mask):
        M = np.einsum("spc,stc->spt", Xs, Xt) * us[:, :, None]
        if mask is not None:
            M = M * mask[None]
        return M

    KAA = cmat(Xa, ua, Xa, strict)
    KAB = cmat(Xa, ua, Xb, None)
    KBB = cmat(Xb, ub, Xb, strict)
    bias_a = 0.5 * KAA.sum(axis=1)                 # [s, 126]
    bias_b = 0.5 * (KAB.sum(axis=1) + KBB.sum(axis=1))

    # within stream fp16 [128, nsb*384]: per sb: KAA | KAB | KBB
    WM = np.zeros((nsb, 3, MP, MP), np.float32)
    WM[:, 0, :TAU, :TAU] = KAA
    WM[:, 1, :TAU, :TAU] = KAB
    WM[:, 2, :TAU, :TAU] = KBB
    WM[:, 0, TAU, :TAU] = -2.0 * bias_a            # * h1row126 (=0.5)
    WM[:, 2, TAU, :TAU] = -2.0 * bias_b
    WM = WM.transpose(2, 0, 1, 3).reshape(MP, nsb * 3 * MP).astype(np.float16)

    # cross stream fp8e4 [128, nsb*512]: per sb s (sources sb s-1):
    # CR[a'->a] | CR[a'->b] | CR[b'->a] | CR[b'->b]   (zero for s=0)
    CRf = np.zeros((nsb, 4, MP, MP), np.float32)
    CRf[1:, 0, :TAU, :TAU] = cmat(Xa[:-1], ua[:-1], Xa[1:], None)
    CRf[1:, 1, :TAU, :TAU] = cmat(Xa[:-1], ua[:-1], Xb[1:], None)
    CRf[1:, 2, :TAU, :TAU] = cmat(Xb[:-1], ub[:-1], Xa[1:], None)
    CRf[1:, 3, :TAU, :TAU] = cmat(Xb[:-1], ub[:-1], Xb[1:], None)
    CR = CRf.transpose(2, 0, 1, 3).reshape(MP, nsb * 4 * MP)
    CR = CR.astype(mybir.dt.np(F8))

    # SW lhsT fp16 [3, nsb*256]: per sb: SW_a | SW_b
    # rows 0:2 = X^T (contracts [W; ones] and, sliced [0:2], the D1 tile),
    # row 2 = +bias (contracts the ones row)
    SW = np.zeros((nsb, 2, 3, MP), np.float32)
    SW[:, 0, 0:2, :TAU] = Xa.transpose(0, 2, 1)
    SW[:, 1, 0:2, :TAU] = Xb.transpose(0, 2, 1)
    SW[:, 0, 2, :TAU] = bias_a
    SW[:, 1, 2, :TAU] = bias_b
    SW = SW.transpose(2, 0, 1, 3).reshape(3, nsb * 2 * MP).astype(np.float16)

    # C stream fp16 [128, nsb*6]: per sb: c_a (3 cols) | c_b (3 cols),
    # third col zero so the D psum [3, 64] keeps row 2 == 0 (ones-row guard)
    C = np.zeros((nsb, MP, 6), np.float32)
    C[:, :TAU, 0:2] = ua[:, :, None] * Xa
    C[:, :TAU, 3:5] = ub[:, :, None] * Xb
    C = C.transpose(1, 0, 2).reshape(MP, nsb * 6).astype(np.float16)

    return WM, CR, SW, C


# ---------------------------------------------------------------------------
# Device program
# ---------------------------------------------------------------------------

def build_nc(nsb: int):
    nc = bacc.Bacc("TRN2", target_bir_lowering=False, debug=False)
    nch = (nsb + CH - 1) // CH
    WM_d = nc.declare_dram_parameter("WM", [MP, nch * CH * 3 * MP], F16,
                                     isOutput=False)
    CR_d = nc.declare_dram_parameter("CR", [MP, nch * CH * 4 * MP], F8,
                                     isOutput=False)
    SW_d = nc.declare_dram_parameter("SW", [3, nch * CH * 2 * MP], F16,
                                     isOutput=False)
    C_d = nc.declare_dram_parameter("C", [MP, nch * CH * 6], F16,
                                    isOutput=False)
    w0_d = nc.declare_dram_parameter("w0T", [3, UH], F32, isOutput=False)
    h2o_d = nc.declare_dram_parameter("h2o", [MP, nsb * MP], F16,
                                      isOutput=True)

    with tile.TileContext(nc) as tc:
        _emit(tc, nc, nsb, nch, WM_d, CR_d, SW_d, C_d, w0_d, h2o_d)
    nc.compile()
    return nc


def _emit(tc, nc, nsb, nch, WM_d, CR_d, SW_d, C_d, w0_d, h2o_d):
    from contextlib import ExitStack
    with ExitStack() as ctx:
        singles = ctx.enter_context(tc.tile_pool(name="singles", bufs=5))
        pool_wm = ctx.enter_context(tc.tile_pool(name="wmc", bufs=3))
        pool_cr = ctx.enter_context(tc.tile_pool(name="crc", bufs=3))
        pool_sw = ctx.enter_context(tc.tile_pool(name="swc", bufs=3))
        pool_c = ctx.enter_context(tc.tile_pool(name="cc", bufs=3))
        pool_h1 = ctx.enter_context(tc.tile_pool(name="h1buf", bufs=3))
        pool_h2 = ctx.enter_context(tc.tile_pool(name="h2buf", bufs=3))
        pool_wh = ctx.enter_context(tc.tile_pool(name="whbuf", bufs=3))
        psum_a = ctx.enter_context(tc.tile_pool(name="psa", bufs=3,
                                                space="PSUM"))
        psum_d = ctx.enter_context(tc.tile_pool(name="psd", bufs=2,
                                                space="PSUM"))
        psum_d1 = ctx.enter_context(tc.tile_pool(name="psd1", bufs=2,
                                                 space="PSUM"))

        w0_sb = singles.tile([3, UH], F32)
        nc.sync.dma_start(out=w0_sb, in_=w0_d[:, :])

        # rhs3: persistent [3, UH] f16 tiles (rows 0:2 = W + D1 combined,
        # row 2 = ones, written once here and never re-written)
        rhs3_0 = singles.tile([3, UH], F16, tag="rhs3_0")
        rhs3_1 = singles.tile([3, UH], F16, tag="rhs3_1")
        rhs3 = [rhs3_0, rhs3_1]
        for r in rhs3:
            nc.vector.tensor_copy(r, w0_sb)

        wh_init = pool_wh.tile([3, UH], F32, tag="wh")
        nc.vector.tensor_copy(wh_init, w0_sb)

        wm_ch, cr_ch, sw_ch, c_ch = {}, {}, {}, {}

        def load_chunk(j):
            if j >= nch:
                return
            wm = pool_wm.tile([MP, CH * 3 * MP], F16, tag="wm")
            nc.sync.dma_start(out=wm, in_=WM_d[:, j * CH * 3 * MP:
                                               (j + 1) * CH * 3 * MP])
            cr = pool_cr.tile([MP, CH * 4 * MP], F8, tag="cr")
            nc.sync.dma_start(out=cr, in_=CR_d[:, j * CH * 4 * MP:
                                               (j + 1) * CH * 4 * MP])
            sw = pool_sw.tile([3, CH * 2 * MP], F16, tag="sw")
            nc.sync.dma_start(out=sw, in_=SW_d[:, j * CH * 2 * MP:
                                               (j + 1) * CH * 2 * MP])
            cc = pool_c.tile([MP, CH * 6], F16, tag="c")
            nc.sync.dma_start(out=cc, in_=C_d[:, j * CH * 6:(j + 1) * CH * 6])
            wm_ch[j], cr_ch[j], sw_ch[j], c_ch[j] = wm, cr, sw, cc

        def wm_ap(s, k):      # k in 0..2: KAA, KAB, KBB
            o = (s % CH) * 3 * MP + k * MP
            return wm_ch[s // CH][:, o:o + MP]

        def cr_ap(s, k):      # k in 0..3
            o = (s % CH) * 4 * MP + k * MP
            return cr_ch[s // CH][:, o:o + MP]

        def sw_ap(s, k):      # k in 0..1
            o = (s % CH) * 2 * MP + k * MP
            return sw_ch[s // CH][:, o:o + MP]

        def c_ap(s, k, w):    # k in 0..1: c_a, c_b; w = 2 (D1) or 3 (D)
            o = (s % CH) * 6 + k * 3
            return c_ch[s // CH][:, o:o + w]

        load_chunk(0)
        load_chunk(1)

        A, H1, H2, D, D1, WH = {}, {}, {}, {}, {}, {}
        WH[-1] = wh_init

        HA = slice(0, UH)          # column half a
        HB = slice(UH, 2 * UH)

        def mm(out, lhsT, rhs, start, stop):
            nc.tensor.matmul(out, lhsT=lhsT, rhs=rhs, start=start, stop=stop,
                             skip_group_check=True)

        def emit_sw(t):
            # A[t] opener + base: SW(t) @ [W+D1; ones].  Exactly ONE
            # start=True per psum tile: a second start on the same tile
            # wipes the first matmul's accumulation (observed on HW); the
            # start resets the whole tile region.
            a = psum_a.tile([MP, 2 * UH], F32, tag="a")
            A[t] = a
            mm(a[:, HA], sw_ap(t, 0), rhs3[t % 2], True, False)
            mm(a[:, HB], sw_ap(t, 1), rhs3[t % 2], False, False)

        def emit_cr(t):
            # cross coupling CR(t-1 -> t) @ h1(t-1); t >= 1
            a = A[t]
            h1p = H1[t - 1]
            mm(a[:, HA], cr_ap(t, 0), h1p[:, HA], False, False)
            mm(a[:, HB], cr_ap(t, 1), h1p[:, HA], False, False)
            mm(a[:, HA], cr_ap(t, 2), h1p[:, HB], False, False)
            mm(a[:, HB], cr_ap(t, 3), h1p[:, HB], False, False)

        def emit_within(s):
            a = A[s]
            h1 = H1[s]
            mm(a[:, HA], wm_ap(s, 0), h1[:, HA], False, True)   # KAA, stop a
            mm(a[:, HB], wm_ap(s, 1), h1[:, HA], False, False)  # KAB
            mm(a[:, HB], wm_ap(s, 2), h1[:, HB], False, True)   # KBB, stop b

        def emit_d1(s):
            d1 = psum_d1.tile([2, UH], F32, tag="d1")
            D1[s] = d1
            mm(d1, c_ap(s, 0, 2), H1[s][:, HA], True, False)
            mm(d1, c_ap(s, 1, 2), H1[s][:, HB], False, True)

        def emit_d(s):
            d = psum_d.tile([3, UH], F32, tag="d")
            D[s] = d
            mm(d, c_ap(s, 0, 3), H2[s][:, HA], True, False)
            mm(d, c_ap(s, 1, 3), H2[s][:, HB], False, True)

        def sig1(s):
            h1 = pool_h1.tile([MP, 2 * UH], F16, tag="h1")
            H1[s] = h1
            nc.scalar.activation(h1, A[s], AF.Sigmoid)

        def sig2(s):
            h2 = pool_h2.tile([MP, 2 * UH], F16, tag="h2")
            H2[s] = h2
            nc.scalar.activation(h2, A[s], AF.Sigmoid)

        def dma_h2(s):
            nc.sync.dma_start(out=h2o_d[:, s * MP:(s + 1) * MP], in_=H2[s])

        # ---- prologue: A[0] = SW(0) only ----
        emit_sw(0)

        EST = 0.00045
        for s in range(nsb):
            tc.tile_set_cur_wait(s * EST)
            if s % CH == 0:
                load_chunk(s // CH + 2)
            # ACT
            sig1(s)
            if s >= 1:
                sig2(s - 1)
                dma_h2(s - 1)
            # PE, ordered for the two critical consumers: CR(s+1) gates
            # sig1(s+1) (the period-critical chain) so it runs right after
            # h1(s); D(s-1) feeds the ledger tail so it runs as soon as
            # h2(s-1) lands; within(s) only gates sig2(s) (a full iter of
            # slack); D1(s) feeds SW(s+2) (also slack).
            if s + 1 < nsb:
                emit_sw(s + 1)
                emit_cr(s + 1)
            emit_d1(s)
            if s >= 1:
                emit_d(s - 1)
            emit_within(s)
            # DVE ledger (lag-3): WH(s) = WH(s-1) + D(s-2) = W(s-1); then
            # rhs3[s % 2] rows 0:2 = WH(s) + D1(s-1) + D1(s), serving
            # SW(s+2) with base W(s-1) + dW(s-1)@h1 + dW(s)@h1.  All
            # inputs except D1(s) come from earlier iterations, so the
            # assembly finishes ~mid-iteration and SW(s+2) never stalls
            # the PE queue head.  Row 2 (ones) is never re-written.
            if s <= 1:
                WH[s] = WH[-1]
            else:
                wh = pool_wh.tile([3, UH], F32, tag="wh")
                nc.vector.tensor_tensor(wh, WH[s - 1], D[s - 2], OP.add)
                WH[s] = wh
            if s + 2 < nsb:
                if s >= 1:
                    nc.vector.tensor_tensor(rhs3[s % 2][0:2, :],
                                            WH[s][0:2, :], D1[s - 1],
                                            OP.add)
                    nc.vector.tensor_tensor(rhs3[s % 2][0:2, :],
                                            rhs3[s % 2][0:2, :], D1[s],
                                            OP.add)
                else:
                    nc.vector.tensor_tensor(rhs3[s % 2][0:2, :],
                                            WH[s][0:2, :], D1[s], OP.add)
            for dlag, store in ((3, A), (3, H1), (3, H2), (2, D), (2, D1),
                                (3, WH)):
                store.pop(s - dlag, None)

        # epilogue
        sig2(nsb - 1)
        dma_h2(nsb - 1)


# ---------------------------------------------------------------------------
# Host wrapper
# ---------------------------------------------------------------------------

def run_cores(X, rewards, W0, nsb, t_real, trace=False):
    WM, CR, SW, C = prep_streams(X, rewards, nsb)
    nch = (nsb + CH - 1) // CH
    cols = {"WM": nch * CH * 3 * MP, "CR": nch * CH * 4 * MP,
            "SW": nch * CH * 2 * MP, "C": nch * CH * 6}

    def pad(a, c):
        if a.shape[1] < c:
            b = np.zeros((a.shape[0], c), a.dtype)
            b[:, :a.shape[1]] = a
            return b
        return a

    WM, CR = pad(WM, cols["WM"]), pad(CR, cols["CR"])
    SW, C = pad(SW, cols["SW"]), pad(C, cols["C"])

    nc = build_nc(nsb)
    in_maps = []
    for c in range(NCORES):
        w0c = np.zeros((3, UH), np.float32)
        w0c[0:2] = W0[c * UH:(c + 1) * UH].T
        w0c[2] = 1.0
        in_maps.append({"WM": WM, "CR": CR, "SW": SW, "C": C, "w0T": w0c})
    res = run_bass_kernel_spmd(nc, in_maps, list(range(NCORES)), trace=trace)
    # h2o rows 0:126 are steps; rows 126:128 are sig(0)=0.5 junk
    total = np.zeros((nsb, SB), np.float64)
    for c in range(NCORES):
        h2o = res.results[c]["h2o"].astype(np.float64)
        per = h2o.reshape(MP, nsb, 2, UH)            # [row, s, tile, unit]
        su = per[:TAU].sum(axis=3)                   # [126, s, 2]
        total += su.transpose(1, 2, 0).reshape(nsb, SB)
    m = (total / float(NH)).reshape(-1)[:t_real].astype(np.float32)
    return m, res


def kernel(X, rewards, W_plastic_init):
    m, _ = run_cores(np.asarray(X, np.float32),
                     np.asarray(rewards, np.float32),
                     np.asarray(W_plastic_init, np.float32),
                     NSB_FULL, T_FULL)
    return m


# revision 29
# speedup vs baseline: 1.0666x; 1.0666x over previous
"""Trainium2 Bass kernel for nn_BehavioralCircuit — v3 superblock solver.

Reference: T=100000 sequential steps of a reward-modulated Hebbian rule over
512 independent 2-D units:
    r[t] = rewards[t] - movavg10(rewards)[t];  u = LR*r
    h    = sigmoid(W @ x_t);  m[t] = h.mean();  W += u[t] * outer(h, x_t)

Scheme (validated against a numpy mirror; full-run rel err ~6e-3):
  Superblocks of SB=252 steps = 2 tiles (a,b) of TAU=126.  Per sb s,
  one PSUM tile A[s] [128, 128] holds both tiles' pre-activations
  (64 units per column half).  Two wide sigmoids per sb:
    h1(s) = sig(X W(s-2)_ledger + X.(c^T h1)(s-2) + CR(s-1->s) h1(s-1)
               + 0.5-seeded within-coupling bias)        [one 128-wide ACT]
    h2(s) = sig(same + K_within (h1(s) - 0.5))           [one 128-wide ACT]
  The 0.5 seed is folded into a host-precomputed bias row of the tiny
  base matmul; its removal at the h2 stage is folded into row 126 of the
  within matrices (h tiles' row 126 is exactly sig(0)=0.5).
  Cross couplings older than one sb are rank-2 (diag(u) X_src X_tgt^T) and
  fold into an f32 weight ledger via tiny c^T h matmuls; only the 3 within
  mats (fp16) and 4 prev-sb cross mats (fp8e4) are streamed from HBM.
  The only tight serial chain is sig1(s-1) -> 4 CR matmuls -> sig1(s).

Per core: 64 units; 8 cores partition the 512 units; host sums unit means.
"""

import sys

import numpy as np

sys.path.insert(0, "/opt/trn_rl_repo")

import concourse.bass as bass
import concourse.bacc as bacc
import concourse.tile as tile
from concourse import mybir
from concourse.bass_utils import run_bass_kernel_spmd

TAU = 126
MP = 128
G = 2
SB = G * TAU            # 252
T_FULL = 100000
NSB_FULL = (T_FULL + SB - 1) // SB   # 397
NB_FULL = NSB_FULL      # test.py compat (loop count = superblocks)
NH = 512
NCORES = 8
UH = NH // NCORES       # 64
CH = 16                 # superblocks per DMA chunk
LR = 0.1
WINDOW = 10

F32 = mybir.dt.float32
F16 = mybir.dt.float16
F8 = mybir.dt.float8e4
AF = mybir.ActivationFunctionType
OP = mybir.AluOpType


# ---------------------------------------------------------------------------
# Host-side stream preparation (shared across cores)
# ---------------------------------------------------------------------------

def _movavg_u(rewards, t_pad):
    cs = np.cumsum(rewards, dtype=np.float64)
    sh = np.concatenate([np.zeros(WINDOW), cs[:-WINDOW]])
    wsum = cs - sh
    count = np.minimum(np.arange(len(rewards)) + 1.0, float(WINDOW))
    u = (LR * (rewards - wsum / count)).astype(np.float32)
    up = np.zeros((t_pad,), np.float32)
    up[: len(rewards)] = u
    return up


def prep_streams(X, rewards, nsb):
    t_pad = nsb * SB
    Xp = np.zeros((t_pad, 2), np.float32)
    Xp[: X.shape[0]] = X
    up = _movavg_u(rewards, t_pad)
    Xa = Xp.reshape(nsb, SB, 2)[:, :TAU]          # [s, 126, 2]
    Xb = Xp.reshape(nsb, SB, 2)[:, TAU:]
    ua = up.reshape(nsb, SB)[:, :TAU]
    ub = up.reshape(nsb, SB)[:, TAU:]

    strict = np.triu(np.ones((TAU, TAU), np.float32), 1)   # [sp, t]: sp < t

    def cmat(Xs, us, Xt, mask):
        M = np.einsum("spc,stc->spt", Xs, Xt) * us[:, :, None]
        if mask is not None:
            M = M * mask[None]
        return M

    KAA = cmat(Xa, ua, Xa, strict)
    KAB = cmat(Xa, ua, Xb, None)
    KBB = cmat(Xb, ub, Xb, strict)
    bias_a = 0.5 * KAA.sum(axis=1)                 # [s, 126]
    bias_b = 0.5 * (KAB.sum(axis=1) + KBB.sum(axis=1))

    # within stream fp16 [128, nsb*384]: per sb: KAA | KAB | KBB
    WM = np.zeros((nsb, 3, MP, MP), np.float32)
    WM[:, 0, :TAU, :TAU] = KAA
    WM[:, 1, :TAU, :TAU] = KAB
    WM[:, 2, :TAU, :TAU] = KBB
    WM[:, 0, TAU, :TAU] = -2.0 * bias_a            # * h1row126 (=0.5)
    WM[:, 2, TAU, :TAU] = -2.0 * bias_b
    WM = WM.transpose(2, 0, 1, 3).reshape(MP, nsb * 3 * MP).astype(np.float16)

    # cross stream fp8e4 [128, nsb*512]: per sb s (sources sb s-1):
    # CR[a'->a] | CR[a'->b] | CR[b'->a] | CR[b'->b]   (zero for s=0)
    CRf = np.zeros((nsb, 4, MP, MP), np.float32)
    CRf[1:, 0, :TAU, :TAU] = cmat(Xa[:-1], ua[:-1], Xa[1:], None)
    CRf[1:, 1, :TAU, :TAU] = cmat(Xa[:-1], ua[:-1], Xb[1:], None)
    CRf[1:, 2, :TAU, :TAU] = cmat(Xb[:-1], ub[:-1], Xa[1:], None)
    CRf[1:, 3, :TAU, :TAU] = cmat(Xb[:-1], ub[:-1], Xb[1:], None)
    CR = CRf.transpose(2, 0, 1, 3).reshape(MP, nsb * 4 * MP)
    CR = CR.astype(mybir.dt.np(F8))

    # SW lhsT fp16 [3, nsb*256]: per sb: SW_a | SW_b
    # rows 0:2 = X^T (contracts [W; ones] and, sliced [0:2], the D1 tile),
    # row 2 = +bias (contracts the ones row)
    SW = np.zeros((nsb, 2, 3, MP), np.float32)
    SW[:, 0, 0:2, :TAU] = Xa.transpose(0, 2, 1)
    SW[:, 1, 0:2, :TAU] = Xb.transpose(0, 2, 1)
    SW[:, 0, 2, :TAU] = bias_a
    SW[:, 1, 2, :TAU] = bias_b
    SW = SW.transpose(2, 0, 1, 3).reshape(3, nsb * 2 * MP).astype(np.float16)

    # C stream fp16 [128, nsb*6]: per sb: c_a (3 cols) | c_b (3 cols),
    # third col zero so the D psum [3, 64] keeps row 2 == 0 (ones-row guard)
    C = np.zeros((nsb, MP, 6), np.float32)
    C[:, :TAU, 0:2] = ua[:, :, None] * Xa
    C[:, :TAU, 3:5] = ub[:, :, None] * Xb
    C = C.transpose(1, 0, 2).reshape(MP, nsb * 6).astype(np.float16)

    return WM, CR, SW, C


# ---------------------------------------------------------------------------
# Device program
# ---------------------------------------------------------------------------

def build_nc(nsb: int):
    nc = bacc.Bacc("TRN2", target_bir_lowering=False, debug=False)
    nch = (nsb + CH - 1) // CH
    WM_d = nc.declare_dram_parameter("WM", [MP, nch * CH * 3 * MP], F16,
                                     isOutput=False)
    CR_d = nc.declare_dram_parameter("CR", [MP, nch * CH * 4 * MP], F8,
                                     isOutput=False)
    SW_d = nc.declare_dram_parameter("SW", [3, nch * CH * 2 * MP], F16,
                                     isOutput=False)
    C_d = nc.declare_dram_parameter("C", [MP, nch * CH * 6], F16,
                                    isOutput=False)
    w0_d = nc.declare_dram_parameter("w0T", [3, UH], F32, isOutput=False)
    h2o_d = nc.declare_dram_parameter("h2o", [MP, nsb * MP], F16,
                                      isOutput=True)

    with tile.TileContext(nc) as tc:
        _emit(tc, nc, nsb, nch, WM_d, CR_d, SW_d, C_d, w0_d, h2o_d)
    nc.compile()
    return nc


def _emit(tc, nc, nsb, nch, WM_d, CR_d, SW_d, C_d, w0_d, h2o_d):
    from contextlib import ExitStack
    with ExitStack() as ctx:
        singles = ctx.enter_context(tc.tile_pool(name="singles", bufs=5))
        pool_wm = ctx.enter_context(tc.tile_pool(name="wmc", bufs=3))
        pool_cr = ctx.enter_context(tc.tile_pool(name="crc", bufs=3))
        pool_sw = ctx.enter_context(tc.tile_pool(name="swc", bufs=3))
        pool_c = ctx.enter_context(tc.tile_pool(name="cc", bufs=3))
        pool_h1 = ctx.enter_context(tc.tile_pool(name="h1buf", bufs=3))
        pool_h2 = ctx.enter_context(tc.tile_pool(name="h2buf", bufs=3))
        pool_wh = ctx.enter_context(tc.tile_pool(name="whbuf", bufs=3))
        psum_a = ctx.enter_context(tc.tile_pool(name="psa", bufs=3,
                                                space="PSUM"))
        psum_d = ctx.enter_context(tc.tile_pool(name="psd", bufs=2,
                                                space="PSUM"))
        psum_d1 = ctx.enter_context(tc.tile_pool(name="psd1", bufs=2,
                                                 space="PSUM"))

        w0_sb = singles.tile([3, UH], F32)
        nc.sync.dma_start(out=w0_sb, in_=w0_d[:, :])

        # rhs3: persistent [3, UH] f16 tiles (rows 0:2 = W + D1 combined,
        # row 2 = ones, written once here and never re-written)
        rhs3_0 = singles.tile([3, UH], F16, tag="rhs3_0")
        rhs3_1 = singles.tile([3, UH], F16, tag="rhs3_1")
        rhs3 = [rhs3_0, rhs3_1]
        for r in rhs3:
            nc.vector.tensor_copy(r, w0_sb)

        wh_init = pool_wh.tile([3, UH], F32, tag="wh")
        nc.vector.tensor_copy(wh_init, w0_sb)

        wm_ch, cr_ch, sw_ch, c_ch = {}, {}, {}, {}

        def load_chunk(j):
            if j >= nch:
                return
            wm = pool_wm.tile([MP, CH * 3 * MP], F16, tag="wm")
            nc.sync.dma_start(out=wm, in_=WM_d[:, j * CH * 3 * MP:
                                               (j + 1) * CH * 3 * MP])
            cr = pool_cr.tile([MP, CH * 4 * MP], F8, tag="cr")
            nc.sync.dma_start(out=cr, in_=CR_d[:, j * CH * 4 * MP:
                                               (j + 1) * CH * 4 * MP])
            sw = pool_sw.tile([3, CH * 2 * MP], F16, tag="sw")
            nc.sync.dma_start(out=sw, in_=SW_d[:, j * CH * 2 * MP:
                                               (j + 1) * CH * 2 * MP])
            cc = pool_c.tile([MP, CH * 6], F16, tag="c")
            nc.sync.dma_start(out=cc, in_=C_d[:, j * CH * 6:(j + 1) * CH * 6])
            wm_ch[j], cr_ch[j], sw_ch[j], c_ch[j] = wm, cr, sw, cc

        def wm_ap(s, k):      # k in 0..2: KAA, KAB, KBB
            o = (s % CH) * 3 * MP + k * MP
            return wm_ch[s // CH][:, o:o + MP]

        def cr_ap(s, k):      # k in 0..3
            o = (s % CH) * 4 * MP + k * MP
            return cr_ch[s // CH][:, o:o + MP]

        def sw_ap(s, k):      # k in 0..1
            o = (s % CH) * 2 * MP + k * MP
            return sw_ch[s // CH][:, o:o + MP]

        def c_ap(s, k, w):    # k in 0..1: c_a, c_b; w = 2 (D1) or 3 (D)
            o = (s % CH) * 6 + k * 3
            return c_ch[s // CH][:, o:o + w]

        load_chunk(0)
        load_chunk(1)

        A, H1, H2, D, D1, WH = {}, {}, {}, {}, {}, {}
        WH[-1] = wh_init

        HA = slice(0, UH)          # column half a
        HB = slice(UH, 2 * UH)

        def mm(out, lhsT, rhs, start, stop):
            nc.tensor.matmul(out, lhsT=lhsT, rhs=rhs, start=start, stop=stop,
                             skip_group_check=True)

        def emit_sw(t):
            # A[t] opener + base: SW(t) @ [W+D1; ones].  Exactly ONE
            # start=True per psum tile: a second start on the same tile
            # wipes the first matmul's accumulation (observed on HW); the
            # start resets the whole tile region.
            a = psum_a.tile([MP, 2 * UH], F32, tag="a")
            A[t] = a
            mm(a[:, HA], sw_ap(t, 0), rhs3[t % 2], True, False)
            mm(a[:, HB], sw_ap(t, 1), rhs3[t % 2], False, False)

        def emit_cr(t):
            # cross coupling CR(t-1 -> t) @ h1(t-1); t >= 1
            a = A[t]
            h1p = H1[t - 1]
            mm(a[:, HA], cr_ap(t, 0), h1p[:, HA], False, False)
            mm(a[:, HB], cr_ap(t, 1), h1p[:, HA], False, False)
            mm(a[:, HA], cr_ap(t, 2), h1p[:, HB], False, False)
            mm(a[:, HB], cr_ap(t, 3), h1p[:, HB], False, False)

        def emit_within(s):
            a = A[s]
            h1 = H1[s]
            mm(a[:, HA], wm_ap(s, 0), h1[:, HA], False, True)   # KAA, stop a
            mm(a[:, HB], wm_ap(s, 1), h1[:, HA], False, False)  # KAB
            mm(a[:, HB], wm_ap(s, 2), h1[:, HB], False, True)   # KBB, stop b

        def emit_d1(s):
            d1 = psum_d1.tile([2, UH], F32, tag="d1")
            D1[s] = d1
            mm(d1, c_ap(s, 0, 2), H1[s][:, HA], True, False)
            mm(d1, c_ap(s, 1, 2), H1[s][:, HB], False, True)

        def emit_d(s):
            d = psum_d.tile([3, UH], F32, tag="d")
            D[s] = d
            mm(d, c_ap(s, 0, 3), H2[s][:, HA], True, False)
            mm(d, c_ap(s, 1, 3), H2[s][:, HB], False, True)

        def sig1(s):
            h1 = pool_h1.tile([MP, 2 * UH], F16, tag="h1")
            H1[s] = h1
            nc.scalar.activation(h1, A[s], AF.Sigmoid)

        def sig2(s):
            h2 = pool_h2.tile([MP, 2 * UH], F16, tag="h2")
            H2[s] = h2
            nc.scalar.activation(h2, A[s], AF.Sigmoid)

        def dma_h2(s):
            nc.sync.dma_start(out=h2o_d[:, s * MP:(s + 1) * MP], in_=H2[s])

        # ---- prologue: A[0] = SW(0) only ----
        emit_sw(0)

        EST = 0.00045
        for s in range(nsb):
            tc.tile_set_cur_wait(s * EST)
            if s % CH == 0:
                load_chunk(s // CH + 2)
            # ACT
            sig1(s)
            if s >= 1:
                sig2(s - 1)
                dma_h2(s - 1)
            # PE, ordered for the two critical consumers: CR(s+1) gates
            # sig1(s+1) (the period-critical chain) so it runs right after
            # h1(s); D(s-1) feeds the ledger tail so it runs as soon as
            # h2(s-1) lands; within(s) only gates sig2(s) (a full iter of
            # slack); D1(s) feeds SW(s+2) (also slack).
            if s + 1 < nsb:
                emit_sw(s + 1)
                emit_cr(s + 1)
            if s >= 1:
                emit_d(s - 1)
            emit_within(s)
            emit_d1(s)
            # DVE ledger: WH(s) = WH(s-1) + D(s-1) (doubles as the eager
            # half of the rhs3 assembly), then rhs3[s % 2] rows 0:2 =
            # WH(s) + D1(s) = W(s) + D1(s), serving SW(s+2); row 2 (ones)
            # is never re-written.
            if s == 0:
                WH[0] = WH[-1]
            else:
                wh = pool_wh.tile([3, UH], F32, tag="wh")
                nc.vector.tensor_tensor(wh, WH[s - 1], D[s - 1], OP.add)
                WH[s] = wh
            if s + 2 < nsb:
                nc.vector.tensor_tensor(rhs3[s % 2][0:2, :],
                                        WH[s][0:2, :], D1[s], OP.add)
            for dlag, store in ((3, A), (3, H1), (3, H2), (2, D), (2, D1),
                                (3, WH)):
                store.pop(s - dlag, None)

        # epilogue
        sig2(nsb - 1)
        dma_h2(nsb - 1)


# ---------------------------------------------------------------------------
# Host wrapper
# ---------------------------------------------------------------------------

def run_cores(X, rewards, W0, nsb, t_real, trace=False):
    WM, CR, SW, C = prep_streams(X, rewards, nsb)
    nch = (nsb + CH - 1) // CH
    cols = {"WM": nch * CH * 3 * MP, "CR": nch * CH * 4 * MP,
            "SW": nch * CH * 2 * MP, "C": nch * CH * 6}

    def pad(a, c):
        if a.shape[1] < c:
            b = np.zeros((a.shape[0], c), a.dtype)
            b[:, :a.shape[1]] = a
            return b
        return a

    WM, CR = pad(WM, cols["WM"]), pad(CR, cols["CR"])
    SW, C = pad(SW, cols["SW"]), pad(C, cols["C"])

    nc = build_nc(nsb)
    in_maps = []
    for c in range(NCORES):
        w0c = np.zeros((3, UH), np.float32)
        w0c[0:2] = W0[c * UH:(c + 1) * UH].T
        w0c[2] = 1.0
        in_maps.append({"WM": WM, "CR": CR, "SW": SW, "C": C, "w0T": w0c})
    res = run_bass_kernel_spmd(nc, in_maps, list(range(NCORES)), trace=trace)
    # h2o rows 0:126 are steps; rows 126:128 are sig(0)=0.5 junk
    total = np.zeros((nsb, SB), np.float64)
    for c in range(NCORES):
        h2o = res.results[c]["h2o"].astype(np.float64)
        per = h2o.reshape(MP, nsb, 2, UH)            # [row, s, tile, unit]
        su = per[:TAU].sum(axis=3)                   # [126, s, 2]
        total += su.transpose(1, 2, 0).reshape(nsb, SB)
    m = (total / float(NH)).reshape(-1)[:t_real].astype(np.float32)
    return m, res


def kernel(X, rewards, W_plastic_init):
    m, _ = run_cores(np.asarray(X, np.float32),
                     np.asarray(rewards, np.float32),
                     np.asarray(W_plastic_init, np.float32),
                     NSB_FULL, T_FULL)
    return m


# revision 35
# speedup vs baseline: 1.1631x; 1.0905x over previous
"""Trainium2 Bass kernel for nn_BehavioralCircuit — v3 superblock solver.

Reference: T=100000 sequential steps of a reward-modulated Hebbian rule over
512 independent 2-D units:
    r[t] = rewards[t] - movavg10(rewards)[t];  u = LR*r
    h    = sigmoid(W @ x_t);  m[t] = h.mean();  W += u[t] * outer(h, x_t)

Scheme (validated against a numpy mirror; full-run rel err ~6e-3):
  Superblocks of SB=252 steps = 2 tiles (a,b) of TAU=126.  Per sb s,
  one PSUM tile A[s] [128, 128] holds both tiles' pre-activations
  (64 units per column half).  Two wide sigmoids per sb:
    h1(s) = sig(X W(s-2)_ledger + X.(c^T h1)(s-2) + CR(s-1->s) h1(s-1)
               + 0.5-seeded within-coupling bias)        [one 128-wide ACT]
    h2(s) = sig(same + K_within (h1(s) - 0.5))           [one 128-wide ACT]
  The 0.5 seed is folded into a host-precomputed bias row of the tiny
  base matmul; its removal at the h2 stage is folded into row 126 of the
  within matrices (h tiles' row 126 is exactly sig(0)=0.5).
  Cross couplings older than one sb are rank-2 (diag(u) X_src X_tgt^T) and
  fold into an f32 weight ledger via tiny c^T h matmuls; only the 3 within
  mats (fp16) and 4 prev-sb cross mats (fp8e4) are streamed from HBM.
  The only tight serial chain is sig1(s-1) -> 4 CR matmuls -> sig1(s).

Per core: 64 units; 8 cores partition the 512 units; host sums unit means.
"""

import sys

import numpy as np

sys.path.insert(0, "/opt/trn_rl_repo")

import concourse.bass as bass
import concourse.bacc as bacc
import concourse.tile as tile
from concourse import mybir
from concourse.bass_utils import run_bass_kernel_spmd

TAU = 126
MP = 128
G = 2
SB = G * TAU            # 252
T_FULL = 100000
NSB_FULL = (T_FULL + SB - 1) // SB   # 397
NB_FULL = NSB_FULL      # test.py compat (loop count = superblocks)
NH = 512
NCORES = 8
UH = NH // NCORES       # 64
CH = 16                 # superblocks per DMA chunk
LR = 0.1
WINDOW = 10

F32 = mybir.dt.float32
F16 = mybir.dt.float16
F8 = mybir.dt.float8e4
AF = mybir.ActivationFunctionType
OP = mybir.AluOpType


# ---------------------------------------------------------------------------
# Host-side stream preparation (shared across cores)
# ---------------------------------------------------------------------------

def _movavg_u(rewards, t_pad):
    cs = np.cumsum(rewards, dtype=np.float64)
    sh = np.concatenate([np.zeros(WINDOW), cs[:-WINDOW]])
    wsum = cs - sh
    count = np.minimum(np.arange(len(rewards)) + 1.0, float(WINDOW))
    u = (LR * (rewards - wsum / count)).astype(np.float32)
    up = np.zeros((t_pad,), np.float32)
    up[: len(rewards)] = u
    return up


def prep_streams(X, rewards, nsb):
    t_pad = nsb * SB
    Xp = np.zeros((t_pad, 2), np.float32)
    Xp[: X.shape[0]] = X
    up = _movavg_u(rewards, t_pad)
    Xa = Xp.reshape(nsb, SB, 2)[:, :TAU]          # [s, 126, 2]
    Xb = Xp.reshape(nsb, SB, 2)[:, TAU:]
    ua = up.reshape(nsb, SB)[:, :TAU]
    ub = up.reshape(nsb, SB)[:, TAU:]

    strict = np.triu(np.ones((TAU, TAU), np.float32), 1)   # [sp, t]: sp < t

    def cmat(Xs, us, Xt, mask):
        M = np.einsum("spc,stc->spt", Xs, Xt) * us[:, :, None]
        if mask is not None:
            M = M * mask[None]
        return M

    KAA = cmat(Xa, ua, Xa, strict)
    KAB = cmat(Xa, ua, Xb, None)
    KBB = cmat(Xb, ub, Xb, strict)
    bias_a = 0.5 * KAA.sum(axis=1)                 # [s, 126]
    bias_b = 0.5 * (KAB.sum(axis=1) + KBB.sum(axis=1))

    # within stream fp16 [128, nsb*384]: per sb: KAA | KAB | KBB
    WM = np.zeros((nsb, 3, MP, MP), np.float32)
    WM[:, 0, :TAU, :TAU] = KAA
    WM[:, 1, :TAU, :TAU] = KAB
    WM[:, 2, :TAU, :TAU] = KBB
    WM[:, 0, TAU, :TAU] = -2.0 * bias_a            # * h1row126 (=0.5)
    WM[:, 2, TAU, :TAU] = -2.0 * bias_b
    WM = WM.transpose(2, 0, 1, 3).reshape(MP, nsb * 3 * MP).astype(np.float16)

    # cross stream fp8e4 [128, nsb*512]: per sb s (sources sb s-1):
    # CR[a'->a] | CR[a'->b] | CR[b'->a] | CR[b'->b]   (zero for s=0)
    CRf = np.zeros((nsb, 4, MP, MP), np.float32)
    CRf[1:, 0, :TAU, :TAU] = cmat(Xa[:-1], ua[:-1], Xa[1:], None)
    CRf[1:, 1, :TAU, :TAU] = cmat(Xa[:-1], ua[:-1], Xb[1:], None)
    CRf[1:, 2, :TAU, :TAU] = cmat(Xb[:-1], ub[:-1], Xa[1:], None)
    CRf[1:, 3, :TAU, :TAU] = cmat(Xb[:-1], ub[:-1], Xb[1:], None)
    CR = CRf.transpose(2, 0, 1, 3).reshape(MP, nsb * 4 * MP)
    CR = CR.astype(mybir.dt.np(F8))

    # SW lhsT fp16 [3, nsb*256]: per sb: SW_a | SW_b
    # rows 0:2 = X^T (contracts [W; ones] and, sliced [0:2], the D1 tile),
    # row 2 = +bias (contracts the ones row)
    SW = np.zeros((nsb, 2, 3, MP), np.float32)
    SW[:, 0, 0:2, :TAU] = Xa.transpose(0, 2, 1)
    SW[:, 1, 0:2, :TAU] = Xb.transpose(0, 2, 1)
    SW[:, 0, 2, :TAU] = bias_a
    SW[:, 1, 2, :TAU] = bias_b
    SW = SW.transpose(2, 0, 1, 3).reshape(3, nsb * 2 * MP).astype(np.float16)

    # C stream fp16 [128, nsb*6]: per sb: c_a (3 cols) | c_b (3 cols),
    # third col zero so the D psum [3, 64] keeps row 2 == 0 (ones-row guard)
    C = np.zeros((nsb, MP, 6), np.float32)
    C[:, :TAU, 0:2] = ua[:, :, None] * Xa
    C[:, :TAU, 3:5] = ub[:, :, None] * Xb
    C = C.transpose(1, 0, 2).reshape(MP, nsb * 6).astype(np.float16)

    return WM, CR, SW, C


# ---------------------------------------------------------------------------
# Device program
# ---------------------------------------------------------------------------

def build_nc(nsb: int):
    nc = bacc.Bacc("TRN2", target_bir_lowering=False, debug=False)
    nch = (nsb + CH - 1) // CH
    WM_d = nc.declare_dram_parameter("WM", [MP, nch * CH * 3 * MP], F16,
                                     isOutput=False)
    CR_d = nc.declare_dram_parameter("CR", [MP, nch * CH * 4 * MP], F8,
                                     isOutput=False)
    SW_d = nc.declare_dram_parameter("SW", [3, nch * CH * 2 * MP], F16,
                                     isOutput=False)
    C_d = nc.declare_dram_parameter("C", [MP, nch * CH * 6], F16,
                                    isOutput=False)
    w0_d = nc.declare_dram_parameter("w0T", [3, UH], F32, isOutput=False)
    h2o_d = nc.declare_dram_parameter("h2o", [MP, nsb * MP], F16,
                                      isOutput=True)

    with tile.TileContext(nc) as tc:
        _emit(tc, nc, nsb, nch, WM_d, CR_d, SW_d, C_d, w0_d, h2o_d)
    nc.compile()
    return nc


def _emit(tc, nc, nsb, nch, WM_d, CR_d, SW_d, C_d, w0_d, h2o_d):
    from contextlib import ExitStack
    with ExitStack() as ctx:
        singles = ctx.enter_context(tc.tile_pool(name="singles", bufs=5))
        pool_wm = ctx.enter_context(tc.tile_pool(name="wmc", bufs=3))
        pool_cr = ctx.enter_context(tc.tile_pool(name="crc", bufs=3))
        pool_sw = ctx.enter_context(tc.tile_pool(name="swc", bufs=3))
        pool_c = ctx.enter_context(tc.tile_pool(name="cc", bufs=3))
        pool_h1 = ctx.enter_context(tc.tile_pool(name="h1buf", bufs=3))
        pool_h2 = ctx.enter_context(tc.tile_pool(name="h2buf", bufs=6))
        pool_wh = ctx.enter_context(tc.tile_pool(name="whbuf", bufs=3))
        psum_a = ctx.enter_context(tc.tile_pool(name="psa", bufs=3,
                                                space="PSUM"))
        psum_d = ctx.enter_context(tc.tile_pool(name="psd", bufs=2,
                                                space="PSUM"))
        psum_d1 = ctx.enter_context(tc.tile_pool(name="psd1", bufs=2,
                                                 space="PSUM"))
        psum_fill = ctx.enter_context(tc.tile_pool(name="psf", bufs=1,
                                                   space="PSUM"))

        w0_sb = singles.tile([3, UH], F32)
        nc.sync.dma_start(out=w0_sb, in_=w0_d[:, :])

        # rhs3: persistent [3, UH] f16 tiles (rows 0:2 = W + D1 combined,
        # row 2 = ones, written once here and never re-written)
        rhs3_0 = singles.tile([3, UH], F16, tag="rhs3_0")
        rhs3_1 = singles.tile([3, UH], F16, tag="rhs3_1")
        rhs3 = [rhs3_0, rhs3_1]
        for r in rhs3:
            nc.vector.tensor_copy(r, w0_sb)

        wh_init = pool_wh.tile([3, UH], F32, tag="wh")
        nc.vector.tensor_copy(wh_init, w0_sb)

        wm_ch, cr_ch, sw_ch, c_ch = {}, {}, {}, {}

        def load_chunk(j):
            if j >= nch:
                return
            wm = pool_wm.tile([MP, CH * 3 * MP], F16, tag="wm")
            nc.sync.dma_start(out=wm, in_=WM_d[:, j * CH * 3 * MP:
                                               (j + 1) * CH * 3 * MP])
            cr = pool_cr.tile([MP, CH * 4 * MP], F8, tag="cr")
            nc.sync.dma_start(out=cr, in_=CR_d[:, j * CH * 4 * MP:
                                               (j + 1) * CH * 4 * MP])
            sw = pool_sw.tile([3, CH * 2 * MP], F16, tag="sw")
            nc.sync.dma_start(out=sw, in_=SW_d[:, j * CH * 2 * MP:
                                               (j + 1) * CH * 2 * MP])
            cc = pool_c.tile([MP, CH * 6], F16, tag="c")
            nc.sync.dma_start(out=cc, in_=C_d[:, j * CH * 6:(j + 1) * CH * 6])
            wm_ch[j], cr_ch[j], sw_ch[j], c_ch[j] = wm, cr, sw, cc

        def wm_ap(s, k):      # k in 0..2: KAA, KAB, KBB
            o = (s % CH) * 3 * MP + k * MP
            return wm_ch[s // CH][:, o:o + MP]

        def cr_ap(s, k):      # k in 0..3
            o = (s % CH) * 4 * MP + k * MP
            return cr_ch[s // CH][:, o:o + MP]

        def sw_ap(s, k):      # k in 0..1
            o = (s % CH) * 2 * MP + k * MP
            return sw_ch[s // CH][:, o:o + MP]

        def c_ap(s, k, w):    # k in 0..1: c_a, c_b; w = 2 (D1) or 3 (D)
            o = (s % CH) * 6 + k * 3
            return c_ch[s // CH][:, o:o + w]

        load_chunk(0)
        load_chunk(1)

        A, H1, H2, D, D1, WH = {}, {}, {}, {}, {}, {}
        WH[-1] = wh_init

        HA = slice(0, UH)          # column half a
        HB = slice(UH, 2 * UH)

        def mm(out, lhsT, rhs, start, stop):
            nc.tensor.matmul(out, lhsT=lhsT, rhs=rhs, start=start, stop=stop,
                             skip_group_check=True)

        def emit_sw(t):
            # A[t] opener + base: SW(t) @ [W+D1; ones].  Exactly ONE
            # start=True per psum tile: a second start on the same tile
            # wipes the first matmul's accumulation (observed on HW); the
            # start resets the whole tile region.
            a = psum_a.tile([MP, 2 * UH], F32, tag="a")
            A[t] = a
            mm(a[:, HA], sw_ap(t, 0), rhs3[t % 2], True, False)
            mm(a[:, HB], sw_ap(t, 1), rhs3[t % 2], False, False)

        def emit_cr(t):
            # cross coupling CR(t-1 -> t) @ h1(t-1); t >= 1
            a = A[t]
            h1p = H1[t - 1]
            mm(a[:, HA], cr_ap(t, 0), h1p[:, HA], False, False)
            mm(a[:, HB], cr_ap(t, 1), h1p[:, HA], False, False)
            mm(a[:, HA], cr_ap(t, 2), h1p[:, HB], False, False)
            mm(a[:, HB], cr_ap(t, 3), h1p[:, HB], False, False)

        fill_tile = psum_fill.tile([MP, UH], F32, tag="fill")

        def emit_fill(s):
            # dependency-free dummy matmul into a scratch psum bank: keeps
            # the tensor engine continuously busy through semaphore-wait
            # gaps so its clock ramps past the mid p-state (the p-state
            # model ramps to 2.4GHz only after ~3us of gapless execution).
            mm(fill_tile, wm_ap(s, 0), wm_ap(s, 1)[:, 0:UH], True, True)

        def emit_within(s):
            a = A[s]
            h1 = H1[s]
            mm(a[:, HA], wm_ap(s, 0), h1[:, HA], False, True)   # KAA, stop a
            mm(a[:, HB], wm_ap(s, 1), h1[:, HA], False, False)  # KAB
            mm(a[:, HB], wm_ap(s, 2), h1[:, HB], False, True)   # KBB, stop b

        def emit_d1(s):
            d1 = psum_d1.tile([2, UH], F32, tag="d1")
            D1[s] = d1
            mm(d1, c_ap(s, 0, 2), H1[s][:, HA], True, False)
            mm(d1, c_ap(s, 1, 2), H1[s][:, HB], False, True)

        def emit_d(s):
            d = psum_d.tile([3, UH], F32, tag="d")
            D[s] = d
            mm(d, c_ap(s, 0, 3), H2[s][:, HA], True, False)
            mm(d, c_ap(s, 1, 3), H2[s][:, HB], False, True)

        def sig1(s):
            h1 = pool_h1.tile([MP, 2 * UH], F16, tag="h1")
            H1[s] = h1
            nc.scalar.activation(h1, A[s], AF.Sigmoid)

        def sig2(s):
            h2 = pool_h2.tile([MP, 2 * UH], F16, tag="h2")
            H2[s] = h2
            nc.scalar.activation(h2, A[s], AF.Sigmoid)

        def dma_h2(s):
            nc.sync.dma_start(out=h2o_d[:, s * MP:(s + 1) * MP], in_=H2[s])

        # ---- prologue: A[0] = SW(0) only ----
        emit_sw(0)

        EST = 0.00045
        for s in range(nsb):
            tc.tile_set_cur_wait(s * EST)
            if s % CH == 0:
                load_chunk(s // CH + 2)
            # ACT
            sig1(s)
            if s >= 1:
                sig2(s - 1)
                dma_h2(s - 1)
            # PE, ordered for the two critical consumers: CR(s+1) gates
            # sig1(s+1) (the period-critical chain) so it runs right after
            # h1(s); D(s-1) feeds the ledger tail so it runs as soon as
            # h2(s-1) lands; within(s) only gates sig2(s) (a full iter of
            # slack); D1(s) feeds SW(s+2) (also slack).
            if s + 1 < nsb:
                emit_sw(s + 1)
                emit_cr(s + 1)
            if s >= 1:
                emit_d(s - 1)
            emit_within(s)
            emit_d1(s)
            # DVE ledger: WH(s) = WH(s-1) + D(s-1) (doubles as the eager
            # half of the rhs3 assembly), then rhs3[s % 2] rows 0:2 =
            # WH(s) + D1(s) = W(s) + D1(s), serving SW(s+2); row 2 (ones)
            # is never re-written.
            if s == 0:
                WH[0] = WH[-1]
            else:
                wh = pool_wh.tile([3, UH], F32, tag="wh")
                nc.vector.tensor_tensor(wh, WH[s - 1], D[s - 1], OP.add)
                WH[s] = wh
            if s + 2 < nsb:
                nc.vector.tensor_tensor(rhs3[s % 2][0:2, :],
                                        WH[s][0:2, :], D1[s], OP.add)
            for dlag, store in ((3, A), (3, H1), (6, H2), (2, D), (2, D1),
                                (3, WH)):
                store.pop(s - dlag, None)

        # epilogue
        sig2(nsb - 1)
        dma_h2(nsb - 1)


# ---------------------------------------------------------------------------
# Host wrapper
# ---------------------------------------------------------------------------

def run_cores(X, rewards, W0, nsb, t_real, trace=False):
    WM, CR, SW, C = prep_streams(X, rewards, nsb)
    nch = (nsb + CH - 1) // CH
    cols = {"WM": nch * CH * 3 * MP, "CR": nch * CH * 4 * MP,
            "SW": nch * CH * 2 * MP, "C": nch * CH * 6}

    def pad(a, c):
        if a.shape[1] < c:
            b = np.zeros((a.shape[0], c), a.dtype)
            b[:, :a.shape[1]] = a
            return b
        return a

    WM, CR = pad(WM, cols["WM"]), pad(CR, cols["CR"])
    SW, C = pad(SW, cols["SW"]), pad(C, cols["C"])

    nc = build_nc(nsb)
    in_maps = []
    for c in range(NCORES):
        w0c = np.zeros((3, UH), np.float32)
        w0c[0:2] = W0[c * UH:(c + 1) * UH].T
        w0c[2] = 1.0
        in_maps.append({"WM": WM, "CR": CR, "SW": SW, "C": C, "w0T": w0c})
    res = run_bass_kernel_spmd(nc, in_maps, list(range(NCORES)), trace=trace)
    # h2o rows 0:126 are steps; rows 126:128 are sig(0)=0.5 junk
    total = np.zeros((nsb, SB), np.float64)
    for c in range(NCORES):
        h2o = res.results[c]["h2o"].astype(np.float64)
        per = h2o.reshape(MP, nsb, 2, UH)            # [row, s, tile, unit]
        su = per[:TAU].sum(axis=3)                   # [126, s, 2]
        total += su.transpose(1, 2, 0).reshape(nsb, SB)
    m = (total / float(NH)).reshape(-1)[:t_real].astype(np.float32)
    return m, res


def kernel(X, rewards, W_plastic_init):
    m, _ = run_cores(np.asarray(X, np.float32),
                     np.asarray(rewards, np.float32),
                     np.asarray(W_plastic_init, np.float32),
                     NSB_FULL, T_FULL)
    return m
